# revision 4
# baseline (speedup 1.0000x reference)
"""DiagonalQuadratic forward: y = sum(Q * x * x, -1) + x @ b + c for x [131072, 512].

Strategy (8-core data parallel, 16384 rows/core):
  y_n = sum_d Q_d x_nd^2 + b_d x_nd + c
      = sum_d sign_d * (s_d x_nd + t_d)^2 + K        (complete the square)
  with s_d = sqrt(|Q_d|), t_d = sign_d b_d / (2 s_d), K = c - sum_d sign_d t_d^2.

Per core, per block of 1024 rows:
  - DMA x block to SBUF in natural layout [128 part, 8*512]
  - PE transpose 128x128 chunks so d lands on partitions (fp32, bit-exact)
  - ACT: z = Square(s_d * x_t + t_d) with per-partition scale/bias, writes f32r
  - PE matmul (f32r, 1 cyc/row): y[1, n] += sign[128,1].T @ z[128, n]
  - DVE adds K, DMA out.

Columns where |Q| is tiny (quadratic-form completion ill-conditioned) are zeroed
on-device and corrected exactly on the host; with the reference distribution
this set is empty.
"""

import sys

if "/opt/trn_rl_repo" not in sys.path:
    sys.path.insert(0, "/opt/trn_rl_repo")

import numpy as np
from contextlib import ExitStack

import concourse.bacc as bacc
import concourse.tile as tile
import concourse.mybir as mybir
from concourse import masks
from concourse.bass_utils import run_bass_kernel_spmd

F32 = mybir.dt.float32
F32R = mybir.dt.float32r

N_TOTAL = 131072
D = 512
N_CORES = 8
N_PC = N_TOTAL // N_CORES       # 16384 rows per core
BLK_N = 1024                    # rows per block
N_BLK = N_PC // BLK_N           # 16 blocks
J = BLK_N // 128                # 8 n-subtiles of 128 rows per block
KCH = D // 128                  # 4 d-chunks
G = BLK_N // 512                # 2 matmul column groups per block

_CACHED_NC = None
_last_prm = None
_last_kc = None


def _build_nc():
    nc = bacc.Bacc("TRN2", target_bir_lowering=False, debug=False, num_devices=N_CORES)
    x_d = nc.dram_tensor("x", [N_PC, D], F32, kind="ExternalInput")
    # packed params: cols 0:4 = s (sqrt|Q|) per d-chunk, 4:8 = t (bias), 8:12 = sign
    prm = nc.dram_tensor("prm", [128, 12], F32, kind="ExternalInput")
    kc = nc.dram_tensor("kc", [1, 1], F32, kind="ExternalInput")
    y_d = nc.dram_tensor("y", [N_BLK, BLK_N], F32, kind="ExternalOutput")

    x_blocks = x_d.ap().rearrange("(a j p) d -> a p j d", j=J, p=128)

    with tile.TileContext(nc) as tc, ExitStack() as ctx:
        cpool = ctx.enter_context(tc.tile_pool(name="cpool", bufs=1))
        xpool = ctx.enter_context(tc.tile_pool(name="xpool", bufs=3))
        zpool = ctx.enter_context(tc.tile_pool(name="zpool", bufs=4))
        opool = ctx.enter_context(tc.tile_pool(name="opool", bufs=2))
        tps = ctx.enter_context(tc.tile_pool(name="tps", bufs=3, space="PSUM"))
        yps = ctx.enter_context(tc.tile_pool(name="yps", bufs=2, space="PSUM"))

        ident = cpool.tile([128, 128], F32)
        masks.make_identity(nc, ident[:])
        prm_sb = cpool.tile([128, 12], F32)
        nc.sync.dma_start(prm_sb[:], prm[:])
        kc_sb = cpool.tile([1, 1], F32)
        nc.sync.dma_start(kc_sb[:], kc[:])
        sgn_r = cpool.tile([128, 4], F32R)
        nc.scalar.copy(sgn_r[:], prm_sb[:, 8:12])

        for blk in range(N_BLK):
            x_sb = xpool.tile([128, J * D], F32)
            nc.sync.dma_start(x_sb[:].rearrange("p (j d) -> p j d", d=D), x_blocks[blk])

            y_ps = yps.tile([1, BLK_N], F32)
            for k in range(KCH):
                for g in range(G):
                    t_ps = tps.tile([128, 512], F32, tag="t_ps")
                    for jj in range(4):
                        j = 4 * g + jj
                        nc.tensor.transpose(
                            t_ps[:, 128 * jj : 128 * (jj + 1)],
                            x_sb[:, j * D + 128 * k : j * D + 128 * (k + 1)],
                            ident[:],
                        )
                    z = zpool.tile([128, 512], F32R, tag="z")
                    nc.scalar.activation(
                        z[:], t_ps[:], mybir.ActivationFunctionType.Square,
                        bias=prm_sb[:, 4 + k : 5 + k], scale=prm_sb[:, k : k + 1],
                    )
                    nc.tensor.matmul(
                        y_ps[0:1, 512 * g : 512 * (g + 1)],
                        sgn_r[:, k : k + 1], z[:],
                        start=(k == 0), stop=(k == KCH - 1),
                    )
            y_sb = opool.tile([1, BLK_N], F32)
            nc.vector.tensor_scalar_add(y_sb[:], y_ps[:], kc_sb[0:1, 0:1])
            nc.sync.dma_start(y_d[blk : blk + 1, :], y_sb[:])

    nc.compile()
    return nc


def kernel(x, Q, b, c):
    global _CACHED_NC
    x = np.ascontiguousarray(x, dtype=np.float32)
    Q64 = np.asarray(Q, dtype=np.float64)
    b64 = np.asarray(b, dtype=np.float64)
    c64 = float(np.asarray(c, dtype=np.float64).reshape(-1)[0])

    absQ = np.abs(Q64)
    # ill-conditioned columns: completion amplifies b^2/(4|Q|); keep device-side
    # values bounded and fix up exactly on host.
    with np.errstate(divide="ignore", invalid="ignore"):
        amp = np.where(absQ > 0, b64 * b64 / (4 * absQ), np.inf)
    bad = (amp > 2000.0) | (absQ == 0.0)

    sgn = np.where(np.asarray(Q) >= 0, 1.0, -1.0).astype(np.float32)
    s = np.sqrt(absQ).astype(np.float32)
    with np.errstate(divide="ignore", invalid="ignore"):
        t = (sgn.astype(np.float64) * b64 / (2 * s.astype(np.float64))).astype(np.float32)
    sgn[bad] = 0.0
    s[bad] = 0.0
    t[bad] = 0.0
    K = np.float32(c64 - np.sum(sgn.astype(np.float64) * t.astype(np.float64) ** 2))

    prm = np.zeros((128, 12), dtype=np.float32)
    prm[:, 0:4] = s.reshape(4, 128).T
    prm[:, 4:8] = t.reshape(4, 128).T
    prm[:, 8:12] = sgn.reshape(4, 128).T
    kc = np.full((1, 1), K, dtype=np.float32)

    global _last_prm, _last_kc
    _last_prm, _last_kc = prm, kc

    if _CACHED_NC is None:
        _CACHED_NC = _build_nc()
    nc = _CACHED_NC

    in_maps = [
        {"x": x[i * N_PC : (i + 1) * N_PC], "prm": prm, "kc": kc}
        for i in range(N_CORES)
    ]
    out = run_bass_kernel_spmd(nc, in_maps, core_ids=list(range(N_CORES)))
    y = np.concatenate([r["y"].reshape(-1) for r in out.results])

    if bad.any():
        idx = np.nonzero(bad)[0]
        xs = x[:, idx].astype(np.float64)
        corr = xs * xs @ Q64[idx] + xs @ b64[idx]
        y = y + corr.astype(np.float32)

    return y.reshape(N_TOTAL, 1).astype(np.float32)


# revision 8
# speedup vs baseline: 12.0821x; 12.0821x over previous
"""DiagonalQuadratic forward: y = sum(Q * x * x, -1) + x @ b + c for x [131072, 512].

Strategy (8-core data parallel, 16384 rows/core):
  y_n = sum_d Q_d x_nd^2 + b_d x_nd + c
      = sum_d sign_d * (s_d x_nd + t_d)^2 + K        (complete the square)
  with s_d = sqrt(|Q_d|), t_d = sign_d b_d / (2 s_d), K = c - sum_d sign_d t_d^2.

Per core, per block of 1024 rows:
  - DMA x block to SBUF [128 part, 8 rows * 512] - each partition holds 8
    consecutive rows = one contiguous 16KB DRAM read per partition
  - PE transpose 128x128 chunks so d lands on partitions (fp32, bit-exact)
  - ACT: z = Square(s_d * x_t + t_d) with per-partition scale/bias -> f32r
  - PE matmul (f32r, 1 cyc/row): y[1, n] += sign[128,1].T @ z[128, n]
  - DVE adds K, GPSIMD DMA out. Host undoes the row permutation on reshape.

Columns where |Q| is tiny (completion ill-conditioned) are zeroed on-device
and corrected exactly on the host (empty set for the reference distribution).
"""

import sys

if "/opt/trn_rl_repo" not in sys.path:
    sys.path.insert(0, "/opt/trn_rl_repo")

import numpy as np
from contextlib import ExitStack

import concourse.bacc as bacc
import concourse.tile as tile
import concourse.mybir as mybir
from concourse import masks
from concourse.bass_utils import run_bass_kernel_spmd

F16 = mybir.dt.float16
F32 = mybir.dt.float32
F32R = mybir.dt.float32r

N_TOTAL = 131072
D = 512
N_CORES = 8
N_PC = N_TOTAL // N_CORES       # 16384 rows per core
BLK_N = 1024                    # rows per block
N_BLK = N_PC // BLK_N           # 16 blocks
R_PP = BLK_N // 128             # consecutive rows per partition per block
KCH = D // 128                  # 4 d-chunks
G = BLK_N // 512                # 2 matmul column groups per block

_CACHED_NC = None
_last_prm = None
_last_kc = None


def _build_nc():
    nc = bacc.Bacc("TRN2", target_bir_lowering=False, debug=False, num_devices=N_CORES)
    x_d = nc.dram_tensor("x", [N_PC, D], F32, kind="ExternalInput")
    # packed params: cols 0:4 = s (sqrt|Q|) per d-chunk, 4:8 = t (bias), 8:12 = sign
    prm = nc.dram_tensor("prm", [128, 12], F32, kind="ExternalInput")
    kc = nc.dram_tensor("kc", [1, 1], F32, kind="ExternalInput")
    y_d = nc.dram_tensor("y", [N_BLK, BLK_N], F32, kind="ExternalOutput")

    # each partition holds R_PP consecutive rows -> one contiguous DRAM read
    # per partition per block
    x_blocks = x_d.ap().rearrange("(a p r) d -> a p r d", p=128, r=R_PP)

    with tile.TileContext(nc) as tc, ExitStack() as ctx:
        cpool = ctx.enter_context(tc.tile_pool(name="cpool", bufs=1))
        xpool = ctx.enter_context(tc.tile_pool(name="xpool", bufs=5))
        zpool = ctx.enter_context(tc.tile_pool(name="zpool", bufs=4))
        opool = ctx.enter_context(tc.tile_pool(name="opool", bufs=2))
        tps = ctx.enter_context(tc.tile_pool(name="tps", bufs=3, space="PSUM"))
        yps = ctx.enter_context(tc.tile_pool(name="yps", bufs=2, space="PSUM"))

        ident = cpool.tile([128, 128], F32)
        masks.make_identity(nc, ident[:])
        prm_sb = cpool.tile([128, 12], F32)
        nc.gpsimd.dma_start(prm_sb[:], prm[:])
        kc_sb = cpool.tile([1, 1], F32)
        nc.gpsimd.dma_start(kc_sb[:], kc[:])
        sgn_r = cpool.tile([128, 4], F32R)
        nc.scalar.copy(sgn_r[:], prm_sb[:, 8:12])

        for blk in range(N_BLK):
            x_sb = xpool.tile([128, R_PP * D], F32)
            nc.sync.dma_start(x_sb[:].rearrange("p (r d) -> p r d", d=D), x_blocks[blk])

            y_ps = yps.tile([1, BLK_N], F32)
            for k in range(KCH):
                for g in range(G):
                    t_ps = tps.tile([128, 512], F32, tag="t_ps")
                    for rr in range(4):
                        r = 4 * g + rr
                        nc.tensor.transpose(
                            t_ps[:, 128 * rr : 128 * (rr + 1)],
                            x_sb[:, r * D + 128 * k : r * D + 128 * (k + 1)],
                            ident[:],
                        )
                    z = zpool.tile([128, 512], F32R, tag="z")
                    nc.scalar.activation(
                        z[:], t_ps[:], mybir.ActivationFunctionType.Square,
                        bias=prm_sb[:, 4 + k : 5 + k], scale=prm_sb[:, k : k + 1],
                    )
                    nc.tensor.matmul(
                        y_ps[0:1, 512 * g : 512 * (g + 1)],
                        sgn_r[:, k : k + 1], z[:],
                        start=(k == 0), stop=(k == KCH - 1),
                    )
            y_sb = opool.tile([1, BLK_N], F32)
            nc.vector.tensor_scalar_add(y_sb[:], y_ps[:], kc_sb[0:1, 0:1])
            nc.gpsimd.dma_start(y_d[blk : blk + 1, :], y_sb[:])

    nc.compile()
    return nc


def kernel(x, Q, b, c):
    global _CACHED_NC
    x32 = np.ascontiguousarray(np.asarray(x, dtype=np.float32))
    Q64 = np.asarray(Q, dtype=np.float64)
    b64 = np.asarray(b, dtype=np.float64)
    c64 = float(np.asarray(c, dtype=np.float64).reshape(-1)[0])

    absQ = np.abs(Q64)
    # ill-conditioned columns: completion amplifies b^2/(4|Q|); keep device-side
    # values bounded and fix up exactly on host.
    with np.errstate(divide="ignore", invalid="ignore"):
        amp = np.where(absQ > 0, b64 * b64 / (4 * absQ), np.inf)
    bad = (amp > 2000.0) | (absQ == 0.0)

    sgn = np.where(np.asarray(Q) >= 0, 1.0, -1.0).astype(np.float32)
    s = np.sqrt(absQ).astype(np.float32)
    with np.errstate(divide="ignore", invalid="ignore"):
        t = (sgn.astype(np.float64) * b64 / (2 * s.astype(np.float64))).astype(np.float32)
    sgn[bad] = 0.0
    s[bad] = 0.0
    t[bad] = 0.0
    K = np.float32(c64 - np.sum(sgn.astype(np.float64) * t.astype(np.float64) ** 2))

    prm = np.zeros((128, 12), dtype=np.float32)
    prm[:, 0:4] = s.reshape(4, 128).T
    prm[:, 4:8] = t.reshape(4, 128).T
    prm[:, 8:12] = sgn.reshape(4, 128).T
    kc = np.full((1, 1), K, dtype=np.float32)

    global _last_prm, _last_kc
    _last_prm, _last_kc = prm, kc

    if _CACHED_NC is None:
        _CACHED_NC = _build_nc()
    nc = _CACHED_NC

    in_maps = [
        {"x": x32[i * N_PC : (i + 1) * N_PC], "prm": prm, "kc": kc}
        for i in range(N_CORES)
    ]
    out = run_bass_kernel_spmd(nc, in_maps, core_ids=list(range(N_CORES)))
    parts = []
    for r in out.results:
        # y_dev[blk, 512*g + 128*rr + p] = y[n0 + R_PP*p + 4*g + rr]
        yb = r["y"].reshape(N_BLK, G, 4, 128)
        parts.append(yb.transpose(0, 3, 1, 2).reshape(-1))
    y = np.concatenate(parts)

    if bad.any():
        idx = np.nonzero(bad)[0]
        xs = x32[:, idx].astype(np.float64)
        corr = (xs * xs) @ Q64[idx] + xs @ b64[idx]
        y = y + corr.astype(np.float32)

    return y.reshape(N_TOTAL, 1).astype(np.float32)


# revision 9
# speedup vs baseline: 12.4852x; 1.0334x over previous
"""DiagonalQuadratic forward: y = sum(Q * x * x, -1) + x @ b + c for x [131072, 512].

Strategy (8-core data parallel, 16384 rows/core):
  y_n = sum_d Q_d x_nd^2 + b_d x_nd + c
      = sum_d sign_d * (s_d x_nd + t_d)^2 + K        (complete the square)
  with s_d = sqrt(|Q_d|), t_d = sign_d b_d / (2 s_d), K = c - sum_d sign_d t_d^2.

Per core, per block of 1024 rows:
  - DMA x block to SBUF [128 part, 8 rows * 512] - each partition holds 8
    consecutive rows = one contiguous 16KB DRAM read per partition
  - PE transpose 128x128 chunks so d lands on partitions (fp32, bit-exact)
  - ACT: z = Square(s_d * x_t + t_d) with per-partition scale/bias -> f32r
  - PE matmul (f32r, 1 cyc/row): y[1, n] += sign[128,1].T @ z[128, n]
  - DVE adds K, GPSIMD DMA out. Host undoes the row permutation on reshape.

Columns where |Q| is tiny (completion ill-conditioned) are zeroed on-device
and corrected exactly on the host (empty set for the reference distribution).
"""

import sys

if "/opt/trn_rl_repo" not in sys.path:
    sys.path.insert(0, "/opt/trn_rl_repo")

import numpy as np
from contextlib import ExitStack

import concourse.bacc as bacc
import concourse.tile as tile
import concourse.mybir as mybir
from concourse import masks
from concourse.bass_utils import run_bass_kernel_spmd

F16 = mybir.dt.float16
F32 = mybir.dt.float32
F32R = mybir.dt.float32r

N_TOTAL = 131072
D = 512
N_CORES = 8
N_PC = N_TOTAL // N_CORES       # 16384 rows per core
BLK_N = 1024                    # rows per block
N_BLK = N_PC // BLK_N           # 16 blocks
R_PP = BLK_N // 128             # consecutive rows per partition per block
KCH = D // 128                  # 4 d-chunks
G = BLK_N // 512                # 2 matmul column groups per block

_CACHED_NC = None
_last_prm = None
_last_kc = None


def _build_nc():
    nc = bacc.Bacc("TRN2", target_bir_lowering=False, debug=False, num_devices=N_CORES)
    x_d = nc.dram_tensor("x", [N_PC, D], F32R, kind="ExternalInput")
    # packed params: cols 0:4 = s (sqrt|Q|) per d-chunk, 4:8 = t (bias), 8:12 = sign
    prm = nc.dram_tensor("prm", [128, 12], F32, kind="ExternalInput")
    kc = nc.dram_tensor("kc", [1, 1], F32, kind="ExternalInput")
    y_d = nc.dram_tensor("y", [N_BLK, BLK_N], F32, kind="ExternalOutput")

    # each partition holds R_PP consecutive rows -> one contiguous DRAM read
    # per partition per block
    x_blocks = x_d.ap().rearrange("(a p r) d -> a p r d", p=128, r=R_PP)

    with tile.TileContext(nc) as tc, ExitStack() as ctx:
        cpool = ctx.enter_context(tc.tile_pool(name="cpool", bufs=1))
        xpool = ctx.enter_context(tc.tile_pool(name="xpool", bufs=5))
        zpool = ctx.enter_context(tc.tile_pool(name="zpool", bufs=4))
        opool = ctx.enter_context(tc.tile_pool(name="opool", bufs=2))
        tps = ctx.enter_context(tc.tile_pool(name="tps", bufs=3, space="PSUM"))
        yps = ctx.enter_context(tc.tile_pool(name="yps", bufs=2, space="PSUM"))

        ident_f = cpool.tile([128, 128], F32)
        masks.make_identity(nc, ident_f[:])
        ident = cpool.tile([128, 128], F32R)
        nc.scalar.copy(ident[:], ident_f[:])
        prm_sb = cpool.tile([128, 12], F32)
        nc.gpsimd.dma_start(prm_sb[:], prm[:])
        kc_sb = cpool.tile([1, 1], F32)
        nc.gpsimd.dma_start(kc_sb[:], kc[:])
        sgn_r = cpool.tile([128, 4], F32R)
        nc.scalar.copy(sgn_r[:], prm_sb[:, 8:12])

        for blk in range(N_BLK):
            x_sb = xpool.tile([128, R_PP * D], F32R)
            nc.sync.dma_start(x_sb[:].rearrange("p (r d) -> p r d", d=D), x_blocks[blk])

            y_ps = yps.tile([1, BLK_N], F32)
            for k in range(KCH):
                for g in range(G):
                    t_ps = tps.tile([128, 512], F32R, tag="t_ps")
                    for rr in range(4):
                        r = 4 * g + rr
                        nc.tensor.transpose(
                            t_ps[:, 128 * rr : 128 * (rr + 1)],
                            x_sb[:, r * D + 128 * k : r * D + 128 * (k + 1)],
                            ident[:],
                        )
                    z = zpool.tile([128, 512], F32R, tag="z")
                    nc.scalar.activation(
                        z[:], t_ps[:], mybir.ActivationFunctionType.Square,
                        bias=prm_sb[:, 4 + k : 5 + k], scale=prm_sb[:, k : k + 1],
                    )
                    nc.tensor.matmul(
                        y_ps[0:1, 512 * g : 512 * (g + 1)],
                        sgn_r[:, k : k + 1], z[:],
                        start=(k == 0), stop=(k == KCH - 1),
                    )
            y_sb = opool.tile([1, BLK_N], F32)
            nc.vector.tensor_scalar_add(y_sb[:], y_ps[:], kc_sb[0:1, 0:1])
            nc.gpsimd.dma_start(y_d[blk : blk + 1, :], y_sb[:])

    nc.compile()
    return nc


def kernel(x, Q, b, c):
    global _CACHED_NC
    x32 = np.ascontiguousarray(np.asarray(x, dtype=np.float32))
    Q64 = np.asarray(Q, dtype=np.float64)
    b64 = np.asarray(b, dtype=np.float64)
    c64 = float(np.asarray(c, dtype=np.float64).reshape(-1)[0])

    absQ = np.abs(Q64)
    # ill-conditioned columns: completion amplifies b^2/(4|Q|); keep device-side
    # values bounded and fix up exactly on host.
    with np.errstate(divide="ignore", invalid="ignore"):
        amp = np.where(absQ > 0, b64 * b64 / (4 * absQ), np.inf)
    bad = (amp > 2000.0) | (absQ == 0.0)

    sgn = np.where(np.asarray(Q) >= 0, 1.0, -1.0).astype(np.float32)
    s = np.sqrt(absQ).astype(np.float32)
    with np.errstate(divide="ignore", invalid="ignore"):
        t = (sgn.astype(np.float64) * b64 / (2 * s.astype(np.float64))).astype(np.float32)
    sgn[bad] = 0.0
    s[bad] = 0.0
    t[bad] = 0.0
    K = np.float32(c64 - np.sum(sgn.astype(np.float64) * t.astype(np.float64) ** 2))

    prm = np.zeros((128, 12), dtype=np.float32)
    prm[:, 0:4] = s.reshape(4, 128).T
    prm[:, 4:8] = t.reshape(4, 128).T
    prm[:, 8:12] = sgn.reshape(4, 128).T
    kc = np.full((1, 1), K, dtype=np.float32)

    global _last_prm, _last_kc
    _last_prm, _last_kc = prm, kc

    if _CACHED_NC is None:
        _CACHED_NC = _build_nc()
    nc = _CACHED_NC

    in_maps = [
        {"x": x32[i * N_PC : (i + 1) * N_PC], "prm": prm, "kc": kc}
        for i in range(N_CORES)
    ]
    out = run_bass_kernel_spmd(nc, in_maps, core_ids=list(range(N_CORES)))
    parts = []
    for r in out.results:
        # y_dev[blk, 512*g + 128*rr + p] = y[n0 + R_PP*p + 4*g + rr]
        yb = r["y"].reshape(N_BLK, G, 4, 128)
        parts.append(yb.transpose(0, 3, 1, 2).reshape(-1))
    y = np.concatenate(parts)

    if bad.any():
        idx = np.nonzero(bad)[0]
        xs = x32[:, idx].astype(np.float64)
        corr = (xs * xs) @ Q64[idx] + xs @ b64[idx]
        y = y + corr.astype(np.float32)

    return y.reshape(N_TOTAL, 1).astype(np.float32)


# revision 10
# speedup vs baseline: 12.7537x; 1.0215x over previous
"""DiagonalQuadratic forward: y = sum(Q * x * x, -1) + x @ b + c for x [131072, 512].

Strategy (8-core data parallel, 16384 rows/core):
  y_n = sum_d Q_d x_nd^2 + b_d x_nd + c
      = sum_d sign_d * (s_d x_nd + t_d)^2 + K        (complete the square)
  with s_d = sqrt(|Q_d|), t_d = sign_d b_d / (2 s_d), K = c - sum_d sign_d t_d^2.

Per core, per block of 1024 rows:
  - DMA x block to SBUF [128 part, 8 rows * 512] - each partition holds 8
    consecutive rows = one contiguous 16KB DRAM read per partition
  - PE transpose 128x128 chunks so d lands on partitions (fp32, bit-exact)
  - ACT: z = Square(s_d * x_t + t_d) with per-partition scale/bias -> f32r
  - PE matmul (f32r, 1 cyc/row): y[1, n] += sign[128,1].T @ z[128, n]
  - DVE adds K, GPSIMD DMA out. Host undoes the row permutation on reshape.

Columns where |Q| is tiny (completion ill-conditioned) are zeroed on-device
and corrected exactly on the host (empty set for the reference distribution).
"""

import sys

if "/opt/trn_rl_repo" not in sys.path:
    sys.path.insert(0, "/opt/trn_rl_repo")

import numpy as np
from contextlib import ExitStack

import concourse.bacc as bacc
import concourse.tile as tile
import concourse.mybir as mybir
from concourse import masks
from concourse.bass_utils import run_bass_kernel_spmd

F16 = mybir.dt.float16
F32 = mybir.dt.float32
F32R = mybir.dt.float32r

N_TOTAL = 131072
D = 512
N_CORES = 8
N_PC = N_TOTAL // N_CORES       # 16384 rows per core
BLK_N = 1024                    # rows per block
N_BLK = N_PC // BLK_N           # 16 blocks
R_PP = BLK_N // 128             # consecutive rows per partition per block
KCH = D // 128                  # 4 d-chunks
G = BLK_N // 512                # 2 matmul column groups per block

_CACHED_NC = None
_last_prm = None
_last_kc = None


def _build_nc():
    nc = bacc.Bacc("TRN2", target_bir_lowering=False, debug=False, num_devices=N_CORES)
    x_d = nc.dram_tensor("x", [N_PC, D], F32R, kind="ExternalInput")
    # packed params: cols 0:4 = s (sqrt|Q|) per d-chunk, 4:8 = t (bias), 8:12 = sign
    prm = nc.dram_tensor("prm", [128, 12], F32, kind="ExternalInput")
    kc = nc.dram_tensor("kc", [1, 1], F32, kind="ExternalInput")
    y_d = nc.dram_tensor("y", [N_BLK, BLK_N], F32, kind="ExternalOutput")

    # each partition holds R_PP consecutive rows -> one contiguous DRAM read
    # per partition per block
    x_blocks = x_d.ap().rearrange("(a p r) d -> a p r d", p=128, r=R_PP)

    with tile.TileContext(nc) as tc, ExitStack() as ctx:
        cpool = ctx.enter_context(tc.tile_pool(name="cpool", bufs=1))
        xpool = ctx.enter_context(tc.tile_pool(name="xpool", bufs=5))
        zpool = ctx.enter_context(tc.tile_pool(name="zpool", bufs=6))
        opool = ctx.enter_context(tc.tile_pool(name="opool", bufs=2))
        tps = ctx.enter_context(tc.tile_pool(name="tps", bufs=4, space="PSUM"))
        yps = ctx.enter_context(tc.tile_pool(name="yps", bufs=2, space="PSUM"))

        ident_f = cpool.tile([128, 128], F32)
        masks.make_identity(nc, ident_f[:])
        ident = cpool.tile([128, 128], F32R)
        nc.scalar.copy(ident[:], ident_f[:])
        prm_sb = cpool.tile([128, 12], F32)
        nc.gpsimd.dma_start(prm_sb[:], prm[:])
        kc_sb = cpool.tile([1, 1], F32)
        nc.gpsimd.dma_start(kc_sb[:], kc[:])
        sgn_r = cpool.tile([128, 4], F32R)
        nc.scalar.copy(sgn_r[:], prm_sb[:, 8:12])

        for blk in range(N_BLK):
            x_sb = xpool.tile([128, R_PP * D], F32R)
            half = R_PP // 2
            for hh in range(2):
                nc.sync.dma_start(
                    x_sb[:, hh * half * D : (hh + 1) * half * D].rearrange(
                        "p (r d) -> p r d", d=D),
                    x_blocks[blk][:, hh * half : (hh + 1) * half],
                )

            y_ps = yps.tile([1, BLK_N], F32)
            for k in range(KCH):
                for g in range(G):
                    t_ps = tps.tile([128, 512], F32R, tag="t_ps")
                    for rr in range(4):
                        r = 4 * g + rr
                        nc.tensor.transpose(
                            t_ps[:, 128 * rr : 128 * (rr + 1)],
                            x_sb[:, r * D + 128 * k : r * D + 128 * (k + 1)],
                            ident[:],
                        )
                    z = zpool.tile([128, 512], F32R, tag="z")
                    nc.scalar.activation(
                        z[:], t_ps[:], mybir.ActivationFunctionType.Square,
                        bias=prm_sb[:, 4 + k : 5 + k], scale=prm_sb[:, k : k + 1],
                    )
                    nc.tensor.matmul(
                        y_ps[0:1, 512 * g : 512 * (g + 1)],
                        sgn_r[:, k : k + 1], z[:],
                        start=(k == 0), stop=(k == KCH - 1),
                    )
            y_sb = opool.tile([1, BLK_N], F32)
            nc.vector.tensor_scalar_add(y_sb[:], y_ps[:], kc_sb[0:1, 0:1])
            nc.gpsimd.dma_start(y_d[blk : blk + 1, :], y_sb[:])

    nc.compile()
    return nc


def kernel(x, Q, b, c):
    global _CACHED_NC
    x32 = np.ascontiguousarray(np.asarray(x, dtype=np.float32))
    Q64 = np.asarray(Q, dtype=np.float64)
    b64 = np.asarray(b, dtype=np.float64)
    c64 = float(np.asarray(c, dtype=np.float64).reshape(-1)[0])

    absQ = np.abs(Q64)
    # ill-conditioned columns: completion amplifies b^2/(4|Q|); keep device-side
    # values bounded and fix up exactly on host.
    with np.errstate(divide="ignore", invalid="ignore"):
        amp = np.where(absQ > 0, b64 * b64 / (4 * absQ), np.inf)
    bad = (amp > 2000.0) | (absQ == 0.0)

    sgn = np.where(np.asarray(Q) >= 0, 1.0, -1.0).astype(np.float32)
    s = np.sqrt(absQ).astype(np.float32)
    with np.errstate(divide="ignore", invalid="ignore"):
        t = (sgn.astype(np.float64) * b64 / (2 * s.astype(np.float64))).astype(np.float32)
    sgn[bad] = 0.0
    s[bad] = 0.0
    t[bad] = 0.0
    K = np.float32(c64 - np.sum(sgn.astype(np.float64) * t.astype(np.float64) ** 2))

    prm = np.zeros((128, 12), dtype=np.float32)
    prm[:, 0:4] = s.reshape(4, 128).T
    prm[:, 4:8] = t.reshape(4, 128).T
    prm[:, 8:12] = sgn.reshape(4, 128).T
    kc = np.full((1, 1), K, dtype=np.float32)

    global _last_prm, _last_kc
    _last_prm, _last_kc = prm, kc

    if _CACHED_NC is None:
        _CACHED_NC = _build_nc()
    nc = _CACHED_NC

    in_maps = [
        {"x": x32[i * N_PC : (i + 1) * N_PC], "prm": prm, "kc": kc}
        for i in range(N_CORES)
    ]
    out = run_bass_kernel_spmd(nc, in_maps, core_ids=list(range(N_CORES)))
    parts = []
    for r in out.results:
        # y_dev[blk, 512*g + 128*rr + p] = y[n0 + R_PP*p + 4*g + rr]
        yb = r["y"].reshape(N_BLK, G, 4, 128)
        parts.append(yb.transpose(0, 3, 1, 2).reshape(-1))
    y = np.concatenate(parts)

    if bad.any():
        idx = np.nonzero(bad)[0]
        xs = x32[:, idx].astype(np.float64)
        corr = (xs * xs) @ Q64[idx] + xs @ b64[idx]
        y = y + corr.astype(np.float32)

    return y.reshape(N_TOTAL, 1).astype(np.float32)
